# revision 17
# baseline (speedup 1.0000x reference)
"""GRU decoder kernel for Trainium2 (Bass/Tile), replicated across 8 NeuronCores.

Problem: 2-layer GRU, HIDDEN=512, BATCH=64, SEQ_LEN=512, feeding its own
layer-2 hidden state back as the next step's input, plus a per-step output
projection to 128 dims.

Why this shape: the axon tunnel to the remote NeuronCores moves ~45-80 MB/s
with ~10-70ms per-transfer latency, while the on-device recurrence takes only
~25ms. So the per-call wall time is dominated by bytes on the wire and
transfer count, and the kernel is organized around minimizing both:
  - The sequence recurrence forces the 3.15M gate weights through the PE
    array every step regardless of batch, so every core runs the identical
    full-batch recurrence (batch/gate-sharding buys nothing).
  - Only core 0's output is ever fetched (one transfer). The output wire
    format is int8 with a per-(step, batch-row) fp32 scale (absmax/127),
    dequantized on the host: 4.3MB instead of 16.8MB fp32. Quantization
    error <= 1 LSB = 7.9e-3 relative, well under the 2e-2 gate.
  - Dispatch is a module-persistent jax.jit(shard_map(bass_exec)) with
    device-resident weights (re-uploaded only if the weight bytes change)
    and non-donated device-resident dummy output operands. Per call only
    the packed initial hidden state (131KB, shard 0 only) goes up.

Kernel internals (unchanged from the tuned baseline):
  - Layout: everything transposed. Hidden state lives as h.T [512,64] packed
    into [128, 256] SBUF tiles (K-tile k at free cols 64k:64k+64). Weights are
    the stationary matmul operand (bf16, full 128-col tiles so the compiler's
    fast-weight-load kicks in); the hidden state is the moving operand. Gates
    land in PSUM as [gate-rows, batch], which is also the right layout for the
    vector-engine gate math (full 128 partitions, contiguous free dim).
  - Single ACT function (Tanh) everywhere: sigmoid(x) = 0.5*tanh(x/2)+0.5,
    algebra folded so no table reloads: with trz = tanh(0.5*(gi+gh+b)),
      v  = (tr + 1) * (h_n + b_hn)            # = 2*r*(h_n+b_hn)
      n  = tanh(i_n + b_in + 0.5*v)
      h' = 0.5*((tz+1)*(h - n)) + n           # = (1-z)*n + z*h
"""

import os
import sys
from concurrent.futures import ThreadPoolExecutor

import numpy as np

sys.path.insert(0, "/opt/trn_rl_repo")

import ml_dtypes  # noqa: E402

BF16 = ml_dtypes.bfloat16

LATENT = 64
H = 512
L = 2
OUT = 128
T = int(os.environ.get("CLAUDE_GRU_T", "512"))
B = 64
P = 128
KT = H // P  # 4 K-tiles
MT = (3 * H) // P  # 12 M-tiles per gate matmul
N_CORES = 8
# devices actually dispatched to; the recurrence is replicated (not sharded),
# so any mesh size >= 1 is numerically identical -- only shard 0 is consumed
N_MESH = int(os.environ.get("CLAUDE_GRU_MESH", "8"))


def _woff(l, m, s, k):
    # free-dim column offset of stationary weight tile (layer, m-tile, src, k-tile)
    return ((((l * MT) + m) * 2 + s) * KT + k) * P


def _pack_T(v):
    # [B, H] -> h.T packed [128, KT*B]: element [p, B*k + b] = v[b, 128k+p]
    assert v.shape == (B, H)
    return (
        v.T.reshape(KT, P, B).transpose(1, 0, 2).reshape(P, KT * B).astype(np.float32)
    )


def _pack_bias(b):
    # [G] (G = 128*g tiles) -> [128, g*B]: [p, B*g + b] = bias[128g+p]
    g = b.shape[0] // P
    return np.repeat(b.reshape(g, P).T[:, :, None], B, axis=2).reshape(P, g * B)


def _build(nc_mod):
    bass, mybir, tile = nc_mod
    from concourse import bacc

    f32 = mybir.dt.float32
    i8 = mybir.dt.int8
    bf16 = mybir.dt.bfloat16
    Tanh = mybir.ActivationFunctionType.Tanh
    add = mybir.AluOpType.add
    mult = mybir.AluOpType.mult
    amax = mybir.AluOpType.max

    nc = bacc.Bacc(
        "TRN2",
        target_bir_lowering=False,
        debug=False,
        enable_asserts=False,
        num_devices=N_CORES,
    )

    wg_d = nc.dram_tensor("wg", [P, L * MT * 2 * KT * P], bf16, kind="ExternalInput")
    bpp_d = nc.dram_tensor("bpp", [P, L * MT], f32, kind="ExternalInput")
    bhn_d = nc.dram_tensor("bhn", [P, L * KT * B], f32, kind="ExternalInput")
    hini_d = nc.dram_tensor("hini", [P, KT * B], f32, kind="ExternalInput")
    wo_d = nc.dram_tensor("wo", [P, KT * OUT], bf16, kind="ExternalInput")
    bo_d = nc.dram_tensor("bo", [B, OUT], f32, kind="ExternalInput")
    # single wire tensor: int8 payload [B, T*OUT] with the per-(step,row)
    # fp32 scales bit-packed into the last 4*T bytes of each row
    out_d = nc.dram_tensor("out", [B, T * OUT + 4 * T], i8, kind="ExternalOutput")
    out_f32 = out_d[:, :].bitcast(f32)  # [B, T*OUT//4 + T] view

    with tile.TileContext(nc) as tc:
        with (
            tc.tile_pool(name="const", bufs=1) as cpool,
            tc.tile_pool(name="state", bufs=1) as spool,
            tc.tile_pool(name="work", bufs=2) as wpool,
            tc.tile_pool(name="psum", bufs=2, space="PSUM") as ppool,
        ):
            wg = cpool.tile([P, L * MT * 2 * KT * P], bf16)
            nc.sync.dma_start(out=wg, in_=wg_d[:, :])
            bpp = cpool.tile([P, L * MT], f32)
            nc.sync.dma_start(out=bpp, in_=bpp_d[:, :])
            bhn = cpool.tile([P, L * KT * B], f32)
            nc.sync.dma_start(out=bhn, in_=bhn_d[:, :])
            wo = cpool.tile([P, KT * OUT], bf16)
            nc.sync.dma_start(out=wo, in_=wo_d[:, :])
            bo = cpool.tile([B, OUT], f32)
            nc.sync.dma_start(out=bo, in_=bo_d[:, :])

            hf = []  # fp32 state, packed h.T
            hb = []  # bf16 copy (matmul moving operand)
            for li in range(L):
                t_f = spool.tile([P, KT * B], f32, tag=f"h{li}f")
                nc.sync.dma_start(out=t_f, in_=hini_d[:, :])
                t_b = spool.tile([P, KT * B], bf16, tag=f"h{li}b")
                nc.vector.tensor_copy(t_b, t_f)
                hf.append(t_f)
                hb.append(t_b)
            xb = spool.tile([P, KT * B], bf16, tag="xb")
            nc.vector.memset(xb, 0.0)

            def gru_layer(li, x_b, h_b, h_f):
                # sources in PSUM-accumulation order; for layer 1 the h-side
                # (available at step start) goes first so PE needn't wait.
                srcs = [(0, x_b), (1, h_b)] if li == 0 else [(1, h_b), (0, x_b)]
                prz = ppool.tile([P, 8 * B], f32, tag="prz")
                pn = ppool.tile([P, 2 * KT * B], f32, tag="pn")
                for m in range(8):
                    first = True
                    for s, src in srcs:
                        for k in range(KT):
                            nc.tensor.matmul(
                                prz[:, B * m : B * (m + 1)],
                                wg[:, _woff(li, m, s, k) : _woff(li, m, s, k) + P],
                                src[:, B * k : B * (k + 1)],
                                start=first,
                                stop=(s == srcs[-1][0] and k == KT - 1),
                            )
                            first = False
                for m in range(KT):
                    for s, src in srcs:
                        half = KT * B if s == 1 else 0
                        for k in range(KT):
                            nc.tensor.matmul(
                                pn[:, half + B * m : half + B * (m + 1)],
                                wg[
                                    :,
                                    _woff(li, 8 + m, s, k) : _woff(li, 8 + m, s, k) + P,
                                ],
                                src[:, B * k : B * (k + 1)],
                                start=(k == 0),
                                stop=(k == KT - 1),
                            )
                # gate math (all fp32)
                abl = os.environ.get("CLAUDE_GRU_ABL", "")
                if abl == "nodve":
                    # timing-diagnostic only: skip gate math, fake h update
                    nc.vector.tensor_copy(h_b, prz[:, : KT * B])
                    return
                # per-subtile tanh with per-partition bias, straight off PSUM:
                #   trz_g = tanh(0.5*prz_g + 0.5*b_rz_g)   (r: g 0..3, z: g 4..7)
                #   n_g   = tanh(w1_g + b_in_g)
                trz = wpool.tile([P, 8 * B], f32, tag="trz")
                for g in range(8):
                    nc.scalar.activation(
                        trz[:, B * g : B * (g + 1)],
                        prz[:, B * g : B * (g + 1)],
                        Tanh,
                        bias=bpp[:, li * MT + g : li * MT + g + 1],
                        scale=0.5,
                    )
                hnb = wpool.tile([P, KT * B], f32, tag="hnb")
                nc.vector.tensor_add(
                    hnb,
                    pn[:, KT * B : 2 * KT * B],
                    bhn[:, li * KT * B : (li + 1) * KT * B],
                )
                v = wpool.tile([P, KT * B], f32, tag="v")
                nc.vector.scalar_tensor_tensor(v, trz[:, : KT * B], 1.0, hnb, add, mult)
                w1 = wpool.tile([P, KT * B], f32, tag="w1")
                nc.vector.scalar_tensor_tensor(w1, v, 0.5, pn[:, : KT * B], mult, add)
                ntl = wpool.tile([P, KT * B], f32, tag="ntl")
                for g in range(KT):
                    nc.scalar.activation(
                        ntl[:, B * g : B * (g + 1)],
                        w1[:, B * g : B * (g + 1)],
                        Tanh,
                        bias=bpp[:, li * MT + 8 + g : li * MT + 8 + g + 1],
                    )
                s1 = wpool.tile([P, KT * B], f32, tag="s1")
                nc.vector.tensor_sub(s1, h_f, ntl)
                q = wpool.tile([P, KT * B], f32, tag="q")
                nc.vector.scalar_tensor_tensor(
                    q, trz[:, KT * B : 2 * KT * B], 1.0, s1, add, mult
                )
                nc.vector.scalar_tensor_tensor(h_f, q, 0.5, ntl, mult, add)
                nc.vector.tensor_copy(h_b, h_f)  # cast fp32 -> bf16

            def step_body(tv):
                gru_layer(0, xb, hb[0], hf[0])
                gru_layer(1, hb[0], hb[1], hf[1])
                nc.gpsimd.tensor_copy(xb, hb[1])  # next step's input (idle engine)
                # output projection: out[b, o] = h1 @ Wo.T + bo
                po = ppool.tile([B, OUT], f32, tag="po")
                for k in range(KT):
                    nc.tensor.matmul(
                        po,
                        hb[1][:, B * k : B * (k + 1)],
                        wo[:, OUT * k : OUT * (k + 1)],
                        start=(k == 0),
                        stop=(k == KT - 1),
                    )
                ob = wpool.tile([B, OUT], f32, tag="ob")
                nc.vector.tensor_add(ob, po, bo)
                # int8 wire quantization: per batch-row absmax scale
                m = wpool.tile([B, 1], f32, tag="m")
                nc.vector.tensor_reduce(
                    m, ob, axis=mybir.AxisListType.X, op=amax,
                    apply_absolute_value=True,
                )
                mc = wpool.tile([B, 1], f32, tag="mc")
                nc.vector.tensor_scalar_max(mc, m, 1e-20)
                r = wpool.tile([B, 1], f32, tag="r")
                nc.vector.reciprocal(r, mc)
                qf = wpool.tile([B, OUT], f32, tag="qf")
                nc.vector.tensor_scalar(qf, ob, r, 127.0, mult, mult)
                obq = wpool.tile([B, OUT], i8, tag="obq")
                nc.vector.tensor_copy(obq, qf)
                nc.sync.dma_start(out=out_d[:, bass.ds(tv * OUT, OUT)], in_=obq)
                nc.sync.dma_start(
                    out=out_f32[:, bass.ds(tv + T * OUT // 4, 1)], in_=mc
                )

            repeat = int(os.environ.get("CLAUDE_GRU_REPEAT", "1"))
            unroll = int(os.environ.get("CLAUDE_GRU_UNROLL", "2"))
            stag = os.environ.get("CLAUDE_GRU_STAG", "1") == "1"
            ET = mybir.EngineType
            loop_kw = dict(
                staggered_reset=stag,
                hint_engines=(ET.PE, ET.DVE, ET.Activation, ET.SP),
            ) if stag else {}
            assert T % unroll == 0

            def run_loop():
                with tc.For_i(0, T, unroll, **loop_kw) as tv:
                    for u in range(unroll):
                        step_body(tv + u if u else tv)

            if repeat > 1:
                # timing-only mode: re-run the whole sequence; output is from
                # the last pass (numerically meaningless, same instruction mix)
                with tc.For_i(0, repeat):
                    run_loop()
            else:
                run_loop()

    nc.compile()
    return nc


_ctx = None


def _get_ctx():
    global _ctx
    if _ctx is None:
        import concourse.bass as bass
        import concourse.mybir as mybir
        import concourse.tile as tile

        nc = _build((bass, mybir, tile))

        import jax
        from concourse import bass2jax, mybir as _mybir
        from jax.experimental.shard_map import shard_map
        from jax.sharding import Mesh, NamedSharding, PartitionSpec

        bass2jax.install_neuronx_cc_hook()

        partition_name = (
            nc.partition_id_tensor.name if nc.partition_id_tensor else None
        )
        in_names = []
        out_names = []
        out_avals = []
        zero_outs = []
        for alloc in nc.m.functions[0].allocations:
            if not isinstance(alloc, _mybir.MemoryLocationSet):
                continue
            name = alloc.memorylocations[0].name
            if alloc.kind == "ExternalInput":
                if name != partition_name:
                    in_names.append(name)
            elif alloc.kind == "ExternalOutput":
                shape = tuple(alloc.tensor_shape)
                dtype = _mybir.dt.np(alloc.dtype)
                out_names.append(name)
                out_avals.append(jax.core.ShapedArray(shape, dtype))
                zero_outs.append(np.zeros(shape, dtype))
        n_params = len(in_names)
        all_names = list(in_names) + list(out_names)
        if partition_name is not None:
            all_names.append(partition_name)

        def _body(*args):
            operands = list(args)
            if partition_name is not None:
                operands.append(bass2jax.partition_id_tensor())
            outs = bass2jax._bass_exec_p.bind(
                *operands,
                out_avals=tuple(out_avals),
                in_names=tuple(all_names),
                out_names=tuple(out_names),
                lowering_input_output_aliases=(),
                sim_require_finite=True,
                sim_require_nnan=True,
                nc=nc,
            )
            return tuple(outs)

        devices = jax.devices()[:N_MESH]
        assert len(devices) == N_MESH
        mesh = Mesh(np.asarray(devices), ("core",))
        in_specs = (PartitionSpec("core"),) * (n_params + len(out_names))
        out_specs = (PartitionSpec("core"),) * len(out_names)
        fn = jax.jit(
            shard_map(
                _body,
                mesh=mesh,
                in_specs=in_specs,
                out_specs=out_specs,
                check_rep=False,
            ),
            keep_unused=True,
        )
        sh = NamedSharding(mesh, PartitionSpec("core"))
        # non-donated device-resident dummy operands for the output slots:
        # uploaded once, reused every call (the kernel writes every element
        # of its outputs, so the uninitialized custom-call result buffers
        # are fully overwritten).
        zeros_dev = [
            jax.device_put(
                np.zeros((N_MESH * z.shape[0], *z.shape[1:]), z.dtype), sh
            )
            for z in zero_outs
        ]
        # per-shard hini buffers: only shard 0 is refreshed each call (cores
        # 1..7 run the same program on stale/zero state; their outputs are
        # never fetched)
        hini_shards = [
            jax.device_put(np.zeros((P, KT * B), np.float32), d) for d in devices
        ]
        _ctx = dict(
            nc=nc,
            fn=fn,
            sh=sh,
            mesh=mesh,
            devices=devices,
            in_names=in_names,
            out_names=out_names,
            zeros=zeros_dev,
            hini_shards=hini_shards,
            jax=jax,
            wraw=None,  # raw weight arrays backing the device cache
            wdev=None,  # name -> device-resident sharded array
        )
    return _ctx


def _pack_weights(W_ih, W_hh, b_ih, b_hh, W_o, b_o):
    wg_np = np.empty((P, L * MT * 2 * KT * P), BF16)
    for li in range(L):
        for s, W in ((0, W_ih[li]), (1, W_hh[li])):
            WT = np.ascontiguousarray(W.T)  # [H, 3H]
            for m in range(MT):
                for k in range(KT):
                    o = _woff(li, m, s, k)
                    wg_np[:, o : o + P] = WT[
                        P * k : P * (k + 1), P * m : P * (m + 1)
                    ].astype(BF16)

    # per-partition bias columns: g<8 -> 0.5*(b_ih+b_hh) for r,z (tanh halves
    # the preactivation, so the ACT bias must be pre-halved); g>=8 -> b_ih n-gate
    bpp_np = np.empty((P, L * MT), np.float32)
    bhn_np = np.empty((P, L * KT * B), np.float32)
    for li in range(L):
        brz = 0.5 * (b_ih[li] + b_hh[li])[: 2 * H]
        bpp_np[:, li * MT : li * MT + 8] = brz.reshape(8, P).T
        bpp_np[:, li * MT + 8 : li * MT + MT] = b_ih[li][2 * H :].reshape(KT, P).T
        bhn_np[:, li * KT * B : (li + 1) * KT * B] = _pack_bias(b_hh[li][2 * H :])

    wo_np = np.ascontiguousarray(W_o.T).astype(BF16).reshape(KT, P, OUT)
    wo_np = wo_np.transpose(1, 0, 2).reshape(P, KT * OUT)
    # (W_o.T is [H, OUT]; k-tile k = rows 128k:128k+128, at free offset 128k)

    bo_np = np.tile(b_o[None, :], (B, 1)).astype(np.float32)
    return {"wg": wg_np, "bpp": bpp_np, "bhn": bhn_np, "wo": wo_np, "bo": bo_np}


def kernel(z, W_l, b_l, W_ih, W_hh, b_ih, b_hh, W_o, b_o):
    z = np.asarray(z, np.float32)
    W_l = np.asarray(W_l, np.float32)
    b_l = np.asarray(b_l, np.float32)
    W_ih = np.asarray(W_ih, np.float32)
    W_hh = np.asarray(W_hh, np.float32)
    b_ih = np.asarray(b_ih, np.float32)
    b_hh = np.asarray(b_hh, np.float32)
    W_o = np.asarray(W_o, np.float32)
    b_o = np.asarray(b_o, np.float32)

    ctx = _get_ctx()
    jax = ctx["jax"]

    # device-resident weight cache, invalidated when the weight bytes change
    wraw = (W_ih, W_hh, b_ih, b_hh, W_o, b_o)
    if ctx["wraw"] is None or not all(
        np.array_equal(a, b) for a, b in zip(ctx["wraw"], wraw)
    ):
        packed = _pack_weights(W_ih, W_hh, b_ih, b_hh, W_o, b_o)
        ctx["wdev"] = {
            k: jax.device_put(np.tile(v, (N_MESH, 1)), ctx["sh"])
            for k, v in packed.items()
        }
        ctx["wraw"] = tuple(np.copy(a) for a in wraw)

    # per-call state: h0 = z @ W_l.T + b_l, packed transposed; only shard 0
    # (core 0) gets the real value -- one small transfer instead of eight --
    # and the device-resident buffer is reused while (z, W_l, b_l) repeat
    hraw = (z, W_l, b_l)
    if ctx.get("hraw") is None or not all(
        np.array_equal(a, b) for a, b in zip(ctx["hraw"], hraw)
    ):
        h0 = z @ W_l.T + b_l  # [B, H]
        ctx["hini_shards"][0] = jax.device_put(_pack_T(h0), ctx["devices"][0])
        ctx["hini_global"] = jax.make_array_from_single_device_arrays(
            (N_MESH * P, KT * B), ctx["sh"], ctx["hini_shards"]
        )
        ctx["hraw"] = tuple(np.copy(a) for a in hraw)
    hini_global = ctx["hini_global"]

    per_name = dict(ctx["wdev"])
    per_name["hini"] = hini_global
    args = [per_name[n] for n in ctx["in_names"]] + ctx["zeros"]

    res = ctx["fn"](*args)
    by_name = dict(zip(ctx["out_names"], res))

    # fetch only core 0's shard: one transfer for payload + packed scales
    shard = by_name["out"].addressable_shards[0].data
    try:
        shard.copy_to_host_async()
    except Exception:
        pass
    raw = np.asarray(shard)  # [B, T*OUT + 4*T] int8
    fq = raw[:, : T * OUT]
    scales = np.ascontiguousarray(raw[:, T * OUT :]).view(np.float32)  # [B, T]
    # dequant in one broadcasted pass (int8 upcasts to f32 inside the ufunc)
    return fq.reshape(B, T, OUT) * (scales * (1.0 / 127.0))[:, :, None]
